# revision 2
# baseline (speedup 1.0000x reference)
"""Trainium2 Bass kernel for nn_CanonicalCov1D (strided dual-projection covariance).

Math (reference):
  shift = W = 128, STRIDE = 8, L = T - 128 = 8064, NWIN = 993
  win1[b,n,:] = X[b, 8n : 8n+128],  win2[b,n,:] = X[b, 128+8n : 256+8n]
  proj_i = win_i @ weight_i  (per (LAT, C))
  cov[b,n,c] = mean_l[(proj1 - mean_l proj1) * (proj2 - mean_l proj2)] + bias

Key simplifications used here:
  * Centering the projections over LAT == projecting with LAT-centered weights,
    so we center (and 1/LAT-scale) the weights on the host and skip the mean
    subtraction on device entirely.
  * win2[n] == win_full[n+16] (shift = 16*STRIDE), so a single im2col of
    1009 windows serves both projections.
  * Weights are laid out c-major ([w, c*32+l]) so the LAT-reduction of the
    elementwise product is a contiguous segmented reduce on the free dim.

Per-core device pipeline (data-parallel over batch, 4 batches/core):
  1. dma_start_transpose builds winT [128(w), 1040(n)] bf16 straight from the
     overlapping-window view of X (xbar transpose, bf16).
  2. For each 128-window chunk and each 32-channel half:
     matmul p1 = winT_chunk^T @ W1c (2x N=512, bf16 -> f32 PSUM)
     matmul p2 = winT_chunk(+16)^T @ W2c
     DVE: p12 = p1 * p2 ; segmented reduce over l -> [128, 32]
  3. bias add (+DMA out) per batch.
"""

import numpy as np

# ---- problem constants (hardcoded; kernel.py must be self-contained) ----
B, T = 32, 8192
W, LAT, C = 128, 32, 64
STRIDE = 8
NWIN = 993            # output windows
NWINF = 1009          # windows incl. +16 shift for proj2
NPAD = 1040           # winT free size (8*128 + 16)
N_CORES = 8
BPC = B // N_CORES    # batches per core
CHUNKS = 8            # ceil(1024/128); chunk 7 has 97 valid output windows
LAST_VALID = NWIN - 7 * 128  # 97

_CACHE = {}


def _build():
    """Build the per-core Bass program. Returns (nc, names)."""
    import concourse.bass as bass
    import concourse.mybir as mybir
    import concourse.tile as tile
    from concourse import bacc

    f32 = mybir.dt.float32
    bf16 = mybir.dt.bfloat16

    nc = bacc.Bacc(
        "TRN2",
        target_bir_lowering=False,
        debug=False,
        enable_asserts=False,
    )

    x_dram = nc.dram_tensor("x", [BPC, T], bf16, kind="ExternalInput")
    w_dram = nc.dram_tensor("w", [W, 2 * LAT * C], bf16, kind="ExternalInput")
    bias_dram = nc.dram_tensor("bias", [128, C], f32, kind="ExternalInput")
    out_dram = nc.dram_tensor("out", [BPC, NWIN, C], f32, kind="ExternalOutput")

    HALF = LAT * C // 2  # 1024 free elems per half (32 c * 32 l)

    with tile.TileContext(nc) as tc:
        with (
            tc.tile_pool(name="consts", bufs=1) as consts,
            tc.tile_pool(name="wins", bufs=2) as wins,
            tc.tile_pool(name="prods", bufs=3) as prods,
            tc.tile_pool(name="outs", bufs=2) as outs,
            tc.tile_pool(name="psum", bufs=2, space="PSUM") as psum,
        ):
            w_sb = consts.tile([W, 2 * LAT * C], bf16)
            nc.sync.dma_start(w_sb[:], w_dram.ap())
            bias_sb = consts.tile([128, C], f32)
            nc.sync.dma_start(bias_sb[:], bias_dram.ap())

            for b in range(BPC):
                winT = wins.tile([128, NPAD], bf16)
                # pad columns (windows 1009..1039 don't exist; zero them so
                # downstream garbage stays finite and is dropped on output)
                nc.vector.memset(winT[:, NWINF:NPAD], 0.0)
                # main im2col transpose: V[n, w] = X[b, 8n + w], n = 0..1007
                v_main = bass.AP(
                    tensor=x_dram, offset=b * T, ap=[[STRIDE, 1008], [1, W]]
                )
                nc.sync.dma_start_transpose(winT[:, 0:1008], v_main)
                # tail windows 993..1008 (16 rows, multiple of XBAR tile)
                v_tail = bass.AP(
                    tensor=x_dram,
                    offset=b * T + 993 * STRIDE,
                    ap=[[STRIDE, 16], [1, W]],
                )
                nc.sync.dma_start_transpose(winT[:, 993:NWINF], v_tail)

                ostage = outs.tile([128, CHUNKS, C], f32)

                for k in range(CHUNKS):
                    lhs1 = winT[:, k * 128 : k * 128 + 128]
                    lhs2 = winT[:, k * 128 + 16 : k * 128 + 144]
                    for h in range(2):
                        p1 = psum.tile([128, HALF], f32, tag="p1")
                        p2 = psum.tile([128, HALF], f32, tag="p2")
                        for q in range(2):
                            nc.tensor.matmul(
                                p1[:, q * 512 : q * 512 + 512],
                                lhs1,
                                w_sb[:, h * HALF + q * 512 : h * HALF + q * 512 + 512],
                                start=True,
                                stop=True,
                            )
                        for q in range(2):
                            nc.tensor.matmul(
                                p2[:, q * 512 : q * 512 + 512],
                                lhs2,
                                w_sb[
                                    :,
                                    2048 + h * HALF + q * 512 : 2048
                                    + h * HALF
                                    + q * 512
                                    + 512,
                                ],
                                start=True,
                                stop=True,
                            )
                        # HW allows only one PSUM read per DVE op: stage p1
                        # through SBUF on the (otherwise idle) scalar engine.
                        p1c = prods.tile([128, HALF], f32, tag="p1c")
                        nc.scalar.copy(p1c[:], p1[:])
                        p12 = prods.tile([128, HALF], f32)
                        nc.vector.tensor_mul(p12[:], p1c[:], p2[:])
                        nc.vector.tensor_reduce(
                            out=ostage[:, k, h * 32 : h * 32 + 32],
                            in_=p12.rearrange("p (c l) -> p c l", l=LAT),
                            axis=mybir.AxisListType.X,
                            op=mybir.AluOpType.add,
                        )

                # bias add (broadcast over chunk dim)
                nc.vector.tensor_add(
                    ostage[:],
                    ostage[:],
                    bias_sb[:, None, :].to_broadcast((128, CHUNKS, C)),
                )

                # store: chunks 0..6 full, chunk 7 first 97 rows
                nc.sync.dma_start(
                    out_dram.ap()[b, 0 : 7 * 128].rearrange(
                        "(k p) c -> p k c", p=128
                    ),
                    ostage[:, 0:7, :],
                )
                nc.sync.dma_start(
                    out_dram.ap()[b, 7 * 128 : NWIN],
                    ostage[0:LAST_VALID, 7, :],
                )

    nc.compile()
    return nc


def _prep_inputs(X, weight1, weight2, bias):
    import ml_dtypes

    X = np.asarray(X, dtype=np.float32)
    weight1 = np.asarray(weight1, dtype=np.float32)
    weight2 = np.asarray(weight2, dtype=np.float32)
    bias = np.asarray(bias, dtype=np.float32)

    # center over LAT, fold 1/LAT into proj1's weights, c-major layout
    w1c = weight1 - weight1.mean(axis=1, keepdims=True)
    w2c = weight2 - weight2.mean(axis=1, keepdims=True)
    w1p = (w1c / LAT).transpose(0, 2, 1).reshape(W, LAT * C)
    w2p = w2c.transpose(0, 2, 1).reshape(W, LAT * C)
    wcat = np.concatenate([w1p, w2p], axis=1).astype(ml_dtypes.bfloat16)

    xb = X.astype(ml_dtypes.bfloat16)
    bias_tiled = np.tile(bias[None, :], (128, 1)).astype(np.float32)

    in_maps = []
    for i in range(N_CORES):
        in_maps.append(
            {
                "x": np.ascontiguousarray(xb[i * BPC : (i + 1) * BPC]),
                "w": wcat,
                "bias": bias_tiled,
            }
        )
    return in_maps


def run_with_results(X, weight1, weight2, bias, trace=False, trace_cores=None):
    from concourse import bass_utils

    if "nc" not in _CACHE:
        _CACHE["nc"] = _build()
    nc = _CACHE["nc"]
    in_maps = _prep_inputs(X, weight1, weight2, bias)
    res = bass_utils.run_bass_kernel_spmd(
        nc,
        in_maps,
        core_ids=list(range(N_CORES)),
        trace=trace,
        trace_cores=trace_cores,
    )
    out = np.concatenate(
        [res.results[i]["out"] for i in range(N_CORES)], axis=0
    ).astype(np.float32)
    return out, res


def kernel(**inputs):
    out, _ = run_with_results(
        inputs["X"], inputs["weight1"], inputs["weight2"], inputs["bias"]
    )
    return out
